# revision 8
# baseline (speedup 1.0000x reference)
"""Trainium2 Bass kernel for nn_HHConv2d_1x1: 255-step Householder cascade
(via compact-WY + log-depth blocked triangular inversion) followed by a 1x1
conv GEMM, data-parallel over batch across 8 NeuronCores.

Self-contained: hardcodes shapes from the problem spec.
  x [32,256,64,64] f32, log_det_jac [32] f32, z [32,16] f32, V [255,256] f32
Returns (y, log_det_jac, z) like the reference.

Math: W = H_1 H_2 ... H_255 with H_i = I - tau_i v_i v_i^T,
      tau_i = 2/(v_i.v_i + eps), v_i = Vfull[i] (ones-padded V rows).
Compact WY:  W = I - Vp^T T Vp  with  T^{-1} = diag(1/tau) + striu(Vp Vp^T).
T is obtained by inverting the upper-triangular R = diag(1/tau) + striu(G)
via 8 levels of X <- X + X*(G.*negmask_k)*X (block inversion recursion,
negated masks fold the sign). The conv is y[b] = W @ x[b] as a
[256]x[256,4096] GEMM per batch, computed in float32r (full PE rate).
"""

import numpy as np

import concourse.bass as bass
import concourse.mybir as mybir
from concourse import bass_utils
from concourse.tile import TileContext
from concourse.tile_sem_assignment import N_PROCS
from concourse.vector_clock import ScopedClock, VectorClock

F32 = mybir.dt.float32
F32R = mybir.dt.float32r
ALU = mybir.AluOpType
AX = mybir.AxisListType

N_CORES = 8
B, C, H, W_ = 32, 256, 64, 64
HW = H * W_          # 4096
BPC = B // N_CORES   # 4 batches per core
EPS = 1e-16
NB_CHUNK = 512       # GEMM free-dim chunk (one fp32 PSUM bank)


class _SplitDrainTileContext(TileContext):
    """The external walrus supports only one embedded sync-wait per
    instruction; hoist the tail drain's waits onto individual SP nops."""

    def _drain_and_barrier(self, tick_clock, wait_clock):
        gc = tick_clock.global_clock
        for p in range(N_PROCS):
            t = gc[p]
            if t <= 0:
                continue
            vec = [0] * N_PROCS
            vec[p] = t
            nop = self.nc.sync.nop(nofuse=True, hint=f"split_drain_wait_{p}")
            wait_clock.add_sem_waits(nop.ins, ScopedClock({None: VectorClock(vec)}))
        self.nc.sync.drain()
        self.nc.all_engine_barrier()
        assert self.sems is not None
        popped = self.nc._tile_sem_poison_stack.pop()
        assert popped is self._sem_poison
        self.nc.clear_and_free_semaphores(list(self.sems.allocated().values()))
        self.nc.all_engine_barrier()


def _split_excess_waits(nc, max_waits=1):
    """Hoist excess per-instruction sync waits onto same-engine NoOps."""
    counter = 0
    for fn in nc.m.functions:
        for bb in fn.blocks:
            insts = bb.instructions
            i = 0
            while i < len(insts):
                inst = insts[i]
                si = inst.sync_info
                if si is not None and len(si.on_wait) > max_waits:
                    waits = list(si.on_wait)
                    si.on_wait = waits[:max_waits]
                    extra = waits[max_waits:]
                    pos = i
                    for j in range(0, len(extra), max_waits):
                        counter += 1
                        noop = mybir.InstNoOp(
                            name=f"I-waitsplit-{counter}", ins=[], outs=[]
                        )
                        noop.engine = inst.engine
                        noop.sync_info = mybir.SyncInfo(
                            on_wait=extra[j : j + max_waits], on_update=[]
                        )
                        insts.insert(pos, noop)
                        pos += 1
                        i += 1
                i += 1


def _make_consts():
    """Input-independent constant tensors."""
    ident = np.eye(C, dtype=np.float32)
    # negated level masks: -1 on the upper-right quadrant of each 2^k-aligned
    # diagonal block
    negmasks = []
    for k in range(1, 9):
        s = 1 << k
        i = np.arange(C)
        same_block = (i[:, None] // s) == (i[None, :] // s)
        upper_right = ((i[:, None] % s) < s // 2) & ((i[None, :] % s) >= s // 2)
        negmasks.append(np.where(same_block & upper_right, -1.0, 0.0).astype(np.float32))
    # Vfull construction masks: row i holds a real vector of size i+2
    # right-aligned; left part is constant 1.0 padding. Row 255 (pad) = 0.
    m = (np.arange(C)[None, :] >= (C - 2 - np.arange(C - 1))[:, None])
    vmask = np.zeros((C, C), np.float32)
    vmask[: C - 1] = m.astype(np.float32)
    vcomp = np.zeros((C, C), np.float32)
    vcomp[: C - 1] = 1.0 - m.astype(np.float32)
    dkeep = np.ones((C, 1), np.float32)
    dkeep[C - 1] = 0.0
    dfix = np.zeros((C, 1), np.float32)
    dfix[C - 1] = 1.0
    return {
        "c_ident": ident,
        **{f"c_negmask{k}": negmasks[k - 1] for k in range(1, 9)},
        "c_vmask": vmask,
        "c_vcomp": vcomp,
        "c_dkeep": dkeep,
        "c_dfix": dfix,
    }


def _load_packed(nc, tile, dram, rows=C):
    """DMA a [rows, C] DRAM tensor into a [128, 2*C] packed tile
    (half h holds global rows h*128.. as partitions, at free offset h*C)."""
    nc.sync.dma_start(tile[:, 0:C], dram[0:128, :])
    if rows > 128:
        nc.sync.dma_start(tile[: rows - 128, C : 2 * C], dram[128:rows, :])


def _build_module():
    nc = bass.Bass()

    x_in = nc.dram_tensor("x", [BPC, C, HW], F32, kind="ExternalInput")
    v_in = nc.dram_tensor("V", [C - 1, C], F32, kind="ExternalInput")
    consts = _make_consts()
    c_handles = {}
    for name, arr in consts.items():
        c_handles[name] = nc.dram_tensor(name, list(arr.shape), F32, kind="ExternalInput")
    y_out = nc.dram_tensor("y", [BPC, C, HW], F32, kind="ExternalOutput")

    with _SplitDrainTileContext(nc) as tc:
        with (
            tc.tile_pool(name="const", bufs=1) as cpool,
            tc.tile_pool(name="work", bufs=1) as wpool,
            tc.tile_pool(name="small", bufs=2) as spool,
            tc.tile_pool(name="xbuf", bufs=8) as xpool,
            tc.tile_pool(name="ybuf", bufs=8) as ypool,
            tc.tile_pool(name="pscasc", bufs=4, space="PSUM") as pscasc,
            tc.tile_pool(name="psgemm", bufs=4, space="PSUM") as psgemm,
        ):
            # ---- constant loads -------------------------------------------------
            ident_t = cpool.tile([128, 2 * C], F32, tag="ident")
            _load_packed(nc, ident_t, c_handles["c_ident"])
            vmask_t = cpool.tile([128, 2 * C], F32, tag="vmask")
            _load_packed(nc, vmask_t, c_handles["c_vmask"])
            vcomp_t = cpool.tile([128, 2 * C], F32, tag="vcomp")
            _load_packed(nc, vcomp_t, c_handles["c_vcomp"])
            negmask_t = []
            for k in range(1, 9):
                t = cpool.tile([128, 2 * C], F32, tag=f"negmask{k}")
                _load_packed(nc, t, c_handles[f"c_negmask{k}"])
                negmask_t.append(t)
            dkeep_t = cpool.tile([128, 1], F32, tag="dkeep")
            nc.sync.dma_start(dkeep_t[:, 0:1], c_handles["c_dkeep"][128:256, :])
            dfix_t = cpool.tile([128, 1], F32, tag="dfix")
            nc.sync.dma_start(dfix_t[:, 0:1], c_handles["c_dfix"][128:256, :])

            # ---- Vp = where(mask, V, 1), padded to 256 rows ---------------------
            vp_t = wpool.tile([128, 2 * C], F32, tag="vp")
            nc.vector.memset(vp_t[:], 0.0)
            nc.sync.dma_start(vp_t[:, 0:C], v_in[0:128, :])
            nc.sync.dma_start(vp_t[0:127, C : 2 * C], v_in[128:255, :])
            nc.vector.tensor_tensor(vp_t[:], vp_t[:], vmask_t[:], ALU.mult)
            nc.vector.tensor_tensor(vp_t[:], vp_t[:], vcomp_t[:], ALU.add)

            # ---- Vt = Vp^T (4 PE block transposes) ------------------------------
            vt_t = wpool.tile([128, 2 * C], F32, tag="vt")
            for hrow in range(2):
                for hcol in range(2):
                    ps = pscasc.tile([128, 128], F32, tag="ps")
                    nc.tensor.transpose(
                        ps[:],
                        vp_t[:, hrow * C + hcol * 128 : hrow * C + (hcol + 1) * 128],
                        ident_t[:, 0:128],
                    )
                    nc.vector.tensor_copy(
                        vt_t[:, hcol * C + hrow * 128 : hcol * C + (hrow + 1) * 128],
                        ps[:],
                    )

            # ---- G = Vp Vp^T ----------------------------------------------------
            g_t = wpool.tile([128, 2 * C], F32, tag="g")
            for mi in range(2):
                ps = pscasc.tile([128, C], F32, tag="ps")
                for kc in range(2):
                    nc.tensor.matmul(
                        ps[:],
                        vt_t[:, kc * C + mi * 128 : kc * C + (mi + 1) * 128],
                        vt_t[:, kc * C : (kc + 1) * C],
                        start=(kc == 0),
                        stop=(kc == 1),
                    )
                nc.vector.tensor_copy(g_t[:, mi * C : (mi + 1) * C], ps[:])

            # ---- recip of R's diagonal: 2/(||v_i||^2 + eps), last entry 1 -------
            sq_t = spool.tile([128, 2 * C], F32, tag="sq")
            nc.vector.tensor_tensor(sq_t[:], vp_t[:], vp_t[:], ALU.mult)
            recip = []
            for hh in range(2):
                d_h = spool.tile([128, 1], F32, tag=f"d{hh}")
                nc.vector.reduce_sum(d_h[:], sq_t[:, hh * C : (hh + 1) * C], AX.X)
                s_h = spool.tile([128, 1], F32, tag=f"s{hh}")
                nc.vector.tensor_scalar(
                    s_h[:], d_h[:], float(EPS), 0.5, ALU.add, ALU.mult
                )
                r_h = spool.tile([128, 1], F32, tag=f"r{hh}")
                nc.vector.reciprocal(r_h[:], s_h[:])
                recip.append(r_h)
            # pad row 255: force recip -> 1
            nc.vector.scalar_tensor_tensor(
                recip[1][:], recip[1][:], dkeep_t[:, 0:1], dfix_t[:], ALU.mult, ALU.add
            )

            # ---- X0 = diag(recip); Z0 = X0 --------------------------------------
            x_t = wpool.tile([128, 2 * C], F32, tag="x")
            z_t = wpool.tile([128, 2 * C], F32, tag="z")
            for hh in range(2):
                nc.vector.tensor_scalar_mul(
                    x_t[:, hh * C : (hh + 1) * C],
                    ident_t[:, hh * C : (hh + 1) * C],
                    recip[hh][:],
                )
            nc.vector.tensor_copy(z_t[:], x_t[:])

            # ---- levels 1..7: X += X E' X within two independent 128-blocks -----
            for k in range(1, 8):
                e_t = spool.tile([128, 2 * C], F32, tag="e")
                nc.vector.tensor_tensor(e_t[:], g_t[:], negmask_t[k - 1][:], ALU.mult)
                for hh in range(2):
                    blk = slice(hh * C + hh * 128, hh * C + hh * 128 + 128)
                    inner_ps = pscasc.tile([128, 128], F32, tag="ps")
                    nc.tensor.matmul(
                        inner_ps[:], e_t[:, blk], z_t[:, blk], start=True, stop=True
                    )
                    inner_sb = spool.tile([128, 128], F32, tag="innersb")
                    nc.vector.tensor_copy(inner_sb[:], inner_ps[:])
                    delta_ps = pscasc.tile([128, 128], F32, tag="ps")
                    nc.tensor.matmul(
                        delta_ps[:], inner_sb[:], x_t[:, blk], start=True, stop=True
                    )
                    nc.vector.tensor_tensor(
                        x_t[:, blk], x_t[:, blk], delta_ps[:], ALU.add
                    )
                if k < 7:
                    # Z = X^T (block transposes of the two diagonal blocks)
                    for hh in range(2):
                        blk = slice(hh * C + hh * 128, hh * C + hh * 128 + 128)
                        ps = pscasc.tile([128, 128], F32, tag="ps")
                        nc.tensor.transpose(ps[:], x_t[:, blk], ident_t[:, 0:128])
                        nc.vector.tensor_copy(z_t[:, blk], ps[:])
                else:
                    # after level 7 only Z's top-left block is still needed
                    ps = pscasc.tile([128, 128], F32, tag="ps")
                    nc.tensor.transpose(ps[:], x_t[:, 0:128], ident_t[:, 0:128])
                    nc.vector.tensor_copy(z_t[:, 0:128], ps[:])

            # ---- level 8: top-right quadrant = A^{-1} E8' C^{-1} ----------------
            e8_t = spool.tile([128, 128], F32, tag="e")
            nc.vector.tensor_tensor(
                e8_t[:], g_t[:, 128:C], negmask_t[7][:, 128:C], ALU.mult
            )
            inner_ps = pscasc.tile([128, 128], F32, tag="ps")
            nc.tensor.matmul(inner_ps[:], e8_t[:], z_t[:, 0:128], start=True, stop=True)
            inner_sb = spool.tile([128, 128], F32, tag="innersb")
            nc.vector.tensor_copy(inner_sb[:], inner_ps[:])
            delta_ps = pscasc.tile([128, 128], F32, tag="ps")
            nc.tensor.matmul(
                delta_ps[:], inner_sb[:], x_t[:, C + 128 : 2 * C], start=True, stop=True
            )
            nc.vector.tensor_tensor(
                x_t[:, 128:C], x_t[:, 128:C], delta_ps[:], ALU.add
            )

            # ---- Wt = I - Vp^T X^T Vp  (= W^T, the GEMM lhsT), in f32r ----------
            n1_t = wpool.tile([128, 2 * C], F32, tag="n1")
            for mi in range(2):
                ps = pscasc.tile([128, C], F32, tag="ps")
                for kc in range(2):
                    nc.tensor.matmul(
                        ps[:],
                        x_t[:, kc * C + mi * 128 : kc * C + (mi + 1) * 128],
                        vp_t[:, kc * C : (kc + 1) * C],
                        start=(kc == 0),
                        stop=(kc == 1),
                    )
                nc.vector.tensor_copy(n1_t[:, mi * C : (mi + 1) * C], ps[:])
            wt_t = wpool.tile([128, 2 * C], F32R, tag="wt")
            for mi in range(2):
                ps = pscasc.tile([128, C], F32, tag="ps")
                for kc in range(2):
                    nc.tensor.matmul(
                        ps[:],
                        vp_t[:, kc * C + mi * 128 : kc * C + (mi + 1) * 128],
                        n1_t[:, kc * C : (kc + 1) * C],
                        start=(kc == 0),
                        stop=(kc == 1),
                    )
                nc.vector.tensor_tensor(
                    wt_t[:, mi * C : (mi + 1) * C],
                    ident_t[:, mi * C : (mi + 1) * C],
                    ps[:],
                    ALU.subtract,
                )

            # ---- GEMM: y[b] = W @ x[b], float32r, streamed in 512-col chunks ----
            for b in range(BPC):
                for nb in range(HW // NB_CHUNK):
                    cs = slice(nb * NB_CHUNK, (nb + 1) * NB_CHUNK)
                    xc = xpool.tile([128, 2 * NB_CHUNK], F32, tag="xc")
                    nc.sync.dma_start(xc[:, 0:NB_CHUNK], x_in[b, 0:128, cs])
                    nc.sync.dma_start(
                        xc[:, NB_CHUNK : 2 * NB_CHUNK], x_in[b, 128:256, cs]
                    )
                    xr = xpool.tile([128, 2 * NB_CHUNK], F32R, tag="xr")
                    nc.vector.tensor_copy(xr[:], xc[:])
                    for mi in range(2):
                        ps = psgemm.tile([128, NB_CHUNK], F32, tag="psy")
                        for kc in range(2):
                            nc.tensor.matmul(
                                ps[:],
                                wt_t[:, kc * C + mi * 128 : kc * C + (mi + 1) * 128],
                                xr[:, kc * NB_CHUNK : (kc + 1) * NB_CHUNK],
                                start=(kc == 0),
                                stop=(kc == 1),
                            )
                        ot = ypool.tile([128, NB_CHUNK], F32, tag="yout")
                        nc.scalar.copy(ot[:], ps[:])
                        nc.sync.dma_start(
                            y_out[b, mi * 128 : (mi + 1) * 128, cs], ot[:]
                        )

    _split_excess_waits(nc, max_waits=1)
    return nc, consts


_CACHE = {}


def _get_module():
    if "nc" not in _CACHE:
        nc, consts = _build_module()
        _CACHE["nc"] = nc
        _CACHE["consts"] = consts
    return _CACHE["nc"], _CACHE["consts"]


def run_on_cores(x, V, trace=False, **kwargs):
    """Run the bass kernel on 8 cores; returns (y_full, BassKernelResults)."""
    nc, consts = _get_module()
    x = np.ascontiguousarray(np.asarray(x, dtype=np.float32)).reshape(B, C, HW)
    V = np.ascontiguousarray(np.asarray(V, dtype=np.float32))
    in_maps = []
    for i in range(N_CORES):
        m = {"x": x[i * BPC : (i + 1) * BPC], "V": V}
        m.update(consts)
        in_maps.append(m)
    res = bass_utils.run_bass_kernel_spmd(
        nc, in_maps, core_ids=list(range(N_CORES)), trace=trace, **kwargs
    )
    y = np.concatenate([res.results[i]["y"] for i in range(N_CORES)], axis=0)
    return y.reshape(B, C, H, W_), res


def kernel(x, log_det_jac, z, V):
    y, _ = run_on_cores(x, V)
    return (
        y,
        np.asarray(log_det_jac, dtype=np.float32).copy(),
        np.asarray(z, dtype=np.float32).copy(),
    )
